# revision 2
# baseline (speedup 1.0000x reference)
"""Trainium2 Bass kernel for AdaptiveNet MLP (fc1+sigmoid, grouped fc2+sigmoid, fc3).

Sharding: pure data-parallel over batch across 8 NeuronCores (no collectives).
Each core computes its 2048-row shard through all three layers.

Layout trick: H1 is permuted s-major on the host (h1' = s*512 + g, where the
original h1 = g*8 + s).  fc1 then produces hT' tiles [128 h1' partitions x 512
rows]; the grouped fc2 contraction over s becomes 8 fused multiply-accumulate
ops on the vector engine with per-partition scalars (W2 columns), and fc3 is a
plain matmul over the 512 groups.  All matmuls run in bf16 with fp32 PSUM
accumulation (rel-err ~5e-3, well inside the 2e-2 gate).
"""

import sys

for _p in ("/opt/trn_rl_repo",):
    if _p not in sys.path:
        sys.path.append(_p)

import numpy as np
import ml_dtypes

BF16 = ml_dtypes.bfloat16

D_IN, H1, H2, D_OUT = 1024, 4096, 512, 256
GS = H1 // H2  # 8
B = 16384
N_CORES = 8
B_SHARD = B // N_CORES  # 2048
NBLK = 512  # rows per block (one PSUM bank of fp32)
NB = B_SHARD // NBLK  # 4
KC = D_IN // 128  # 8 contraction chunks for fc1
CC = H1 // 128  # 32 h1' chunks
NT = H2 // 128  # 4 x2T tiles
ND = D_OUT // 128  # 2 output chunks

_compiled = {}


def _build_nc():
    from concourse import bacc, tile, mybir

    f32 = mybir.dt.float32
    bf16 = mybir.dt.bfloat16
    AF = mybir.ActivationFunctionType
    ALU = mybir.AluOpType

    nc = bacc.Bacc("TRN2", target_bir_lowering=False, debug=False,
                   num_devices=N_CORES)

    xt = nc.dram_tensor("xt", [D_IN, B_SHARD], bf16, kind="ExternalInput")
    w1t = nc.dram_tensor("w1t", [D_IN, H1], bf16, kind="ExternalInput")
    w2c = nc.dram_tensor("w2c", [128, CC], f32, kind="ExternalInput")
    b1c = nc.dram_tensor("b1c", [128, CC], f32, kind="ExternalInput")
    b2c = nc.dram_tensor("b2c", [128, NT], f32, kind="ExternalInput")
    w3t = nc.dram_tensor("w3t", [H2, D_OUT], bf16, kind="ExternalInput")
    b3c = nc.dram_tensor("b3c", [128, ND], f32, kind="ExternalInput")
    out = nc.dram_tensor("out", [D_OUT, B_SHARD], f32, kind="ExternalOutput")

    with tile.TileContext(nc) as tc:
        with (
            tc.tile_pool(name="wpool", bufs=1) as wpool,
            tc.tile_pool(name="xpool", bufs=2) as xpool,
            tc.tile_pool(name="hpool", bufs=6) as hpool,
            tc.tile_pool(name="accpool", bufs=2) as accpool,
            tc.tile_pool(name="x2pool", bufs=2) as x2pool,
            tc.tile_pool(name="opool", bufs=3) as opool,
            tc.tile_pool(name="psum_h", bufs=4, space="PSUM") as psum_h_pool,
            tc.tile_pool(name="psum_o", bufs=2, space="PSUM") as psum_o_pool,
        ):
            # --- persistent weights/constants ---
            w1_sb = []
            for k in range(KC):
                t = wpool.tile([128, H1], bf16, tag=f"w1_{k}", name=f"w1sb_{k}")
                nc.sync.dma_start(t[:], w1t.ap()[128 * k:128 * (k + 1), :])
                w1_sb.append(t)
            w3_sb = []
            for t_i in range(NT):
                t = wpool.tile([128, D_OUT], bf16, tag=f"w3_{t_i}", name=f"w3sb_{t_i}")
                nc.sync.dma_start(t[:], w3t.ap()[128 * t_i:128 * (t_i + 1), :])
                w3_sb.append(t)
            w2_sb = wpool.tile([128, CC], f32, tag="w2c")
            nc.sync.dma_start(w2_sb[:], w2c.ap()[:])
            b1_sb = wpool.tile([128, CC], f32, tag="b1c")
            nc.sync.dma_start(b1_sb[:], b1c.ap()[:])
            b2_sb = wpool.tile([128, NT], f32, tag="b2c")
            nc.sync.dma_start(b2_sb[:], b2c.ap()[:])
            b3_sb = wpool.tile([128, ND], f32, tag="b3c")
            nc.sync.dma_start(b3_sb[:], b3c.ap()[:])

            for n in range(NB):
                n0 = n * NBLK
                # load x block: 8 k-tiles [128, NBLK]
                x_sb = []
                for k in range(KC):
                    t = xpool.tile([128, NBLK], bf16, tag=f"x_{k}", name=f"xsb_{n}_{k}")
                    nc.sync.dma_start(
                        t[:], xt.ap()[128 * k:128 * (k + 1), n0:n0 + NBLK])
                    x_sb.append(t)

                acc = [None] * NT
                for c in range(CC):
                    ph = psum_h_pool.tile([128, NBLK], f32, tag="psum_h", name=f"ph_{n}_{c}")
                    for k in range(KC):
                        nc.tensor.matmul(
                            ph[:],
                            lhsT=w1_sb[k][:, 128 * c:128 * (c + 1)],
                            rhs=x_sb[k][:],
                            start=(k == 0),
                            stop=(k == KC - 1),
                        )
                    ht = hpool.tile([128, NBLK], bf16, tag="ht", name=f"ht_{n}_{c}")
                    nc.scalar.activation(ht[:], ph[:], AF.Sigmoid,
                                         bias=b1_sb[:, c:c + 1])
                    t_i = c % NT
                    if c < NT:
                        acc[t_i] = accpool.tile([128, NBLK], f32,
                                                tag=f"acc_{t_i}",
                                                name=f"acc_{n}_{t_i}")
                        nc.vector.tensor_scalar_mul(acc[t_i][:], ht[:],
                                                    w2_sb[:, c:c + 1])
                    else:
                        nc.vector.scalar_tensor_tensor(
                            acc[t_i][:], ht[:], w2_sb[:, c:c + 1],
                            acc[t_i][:], op0=ALU.mult, op1=ALU.add)

                x2_sb = []
                for t_i in range(NT):
                    t = x2pool.tile([128, NBLK], bf16, tag=f"x2_{t_i}", name=f"x2sb_{n}_{t_i}")
                    nc.scalar.activation(t[:], acc[t_i][:], AF.Sigmoid,
                                         bias=b2_sb[:, t_i:t_i + 1])
                    x2_sb.append(t)

                for d in range(ND):
                    po = psum_o_pool.tile([128, NBLK], f32, tag="psum_o", name=f"po_{n}_{d}")
                    for t_i in range(NT):
                        nc.tensor.matmul(
                            po[:],
                            lhsT=w3_sb[t_i][:, 128 * d:128 * (d + 1)],
                            rhs=x2_sb[t_i][:],
                            start=(t_i == 0),
                            stop=(t_i == NT - 1),
                        )
                    ot = opool.tile([128, NBLK], f32, tag="ot", name=f"ot_{n}_{d}")
                    nc.scalar.activation(ot[:], po[:], AF.Identity,
                                         bias=b3_sb[:, d:d + 1])
                    nc.sync.dma_start(
                        out.ap()[128 * d:128 * (d + 1), n0:n0 + NBLK], ot[:])

    nc.compile()
    return nc


def get_nc():
    if "nc" not in _compiled:
        _compiled["nc"] = _build_nc()
    return _compiled["nc"]


def make_in_maps(x, W1, b1, W2, b2, W3, b3):
    x = np.asarray(x, dtype=np.float32)
    W1 = np.asarray(W1, dtype=np.float32)
    b1 = np.asarray(b1, dtype=np.float32)
    W2 = np.asarray(W2, dtype=np.float32)
    b2 = np.asarray(b2, dtype=np.float32)
    W3 = np.asarray(W3, dtype=np.float32)
    b3 = np.asarray(b3, dtype=np.float32)

    # s-major permutation of H1: new index p = s*H2 + g  (old h1 = g*GS + s)
    p = np.arange(H1)
    perm = (p % H2) * GS + (p // H2)
    W1p = W1[perm, :]
    b1p = b1[perm]

    w1t_h = np.ascontiguousarray(W1p.T).astype(BF16)  # [D_IN, H1]
    b1c_h = np.ascontiguousarray(b1p.reshape(CC, 128).T, dtype=np.float32)
    # chunk c: s = c//NT, tile t = c%NT, partition k <-> group 128*t + k
    w2c_h = np.empty((128, CC), dtype=np.float32)
    for c in range(CC):
        w2c_h[:, c] = W2[128 * (c % NT):128 * (c % NT) + 128, c // NT]
    b2c_h = np.ascontiguousarray(b2.reshape(NT, 128).T, dtype=np.float32)
    w3t_h = np.ascontiguousarray(W3.T).astype(BF16)  # [H2, D_OUT]
    b3c_h = np.ascontiguousarray(b3.reshape(ND, 128).T, dtype=np.float32)

    xt_h = x.T.astype(BF16)  # [D_IN, B]

    in_maps = []
    for i in range(N_CORES):
        in_maps.append({
            "xt": np.ascontiguousarray(
                xt_h[:, i * B_SHARD:(i + 1) * B_SHARD]),
            "w1t": w1t_h,
            "w2c": w2c_h,
            "b1c": b1c_h,
            "b2c": b2c_h,
            "w3t": w3t_h,
            "b3c": b3c_h,
        })
    return in_maps


def kernel(x, W1, b1, W2, b2, W3, b3):
    from concourse.bass_utils import run_bass_kernel_spmd

    nc = get_nc()
    in_maps = make_in_maps(x, W1, b1, W2, b2, W3, b3)
    res = run_bass_kernel_spmd(nc, in_maps, core_ids=list(range(N_CORES)))
    outT = np.concatenate([res.results[i]["out"] for i in range(N_CORES)],
                          axis=1)  # [D_OUT, B]
    return np.ascontiguousarray(outT.T)
